# revision 15
# baseline (speedup 1.0000x reference)
"""ForwardDiffusion (Ornstein-Uhlenbeck Euler-Maruyama) Trainium2 kernel.

Math: x_k = a*x_{k-1} + b*z_k with a = 1-THETA*DT, b = SIGMA0*sqrt(DT).
Closed form: x_k = a^k * x0 + c_k where c_k = sum_{j<=k} a^{k-j} * b * z_j
depends only on the (batch-shared) noise. Each core handles 8 batch rows:
  - c tiles (k blocks of 128 on partitions) via triangular matmul on PE,
    with rank-1 carry propagation between blocks (also PE).
  - out tile (b, kb) = (xb_bcast * a^k) + c  in ONE fused DVE op,
    then streamed to DRAM. Memory-bound: 32MB written per core.
Data parallel over batch: x sharded 8 ways, noise replicated, no collectives.
"""

import math
import os

import numpy as np

import concourse.bass as bass
import concourse.bacc as bacc
import concourse.mybir as mybir
import concourse.tile as tile
from concourse.bass_utils import run_bass_kernel_spmd

# Problem config (hardcoded per harness contract)
THETA = 1.0
SIGMA0 = 0.5
DT = 0.001
BATCH = 64
LENGTH = 1024
STEPS = 1000          # output rows per batch element (k = 0..999)
NK = STEPS - 1        # noise rows (k = 1..999)
NCORES = 8
BPC = BATCH // NCORES  # batch rows per core = 8
NKB = (NK + 127) // 128  # 8 k-blocks (last one has 103 rows)

A = 1.0 - THETA * DT            # 0.999
B = SIGMA0 * math.sqrt(DT)      # 0.0158113883...

F32 = mybir.dt.float32

_cache = {}


def _consts():
    """Host-precomputed constant tensors (exact in f64, cast to f32)."""
    if "consts" in _cache:
        return _cache["consts"]
    p = np.arange(128, dtype=np.float64)
    # lhsT for in-block triangular matmul: mtriT[q, p] = b * a^(p-q) for q <= p
    pq = p[None, :] - p[:, None]           # [q, p] -> p - q
    mtriT = np.where(pq >= 0, B * A ** pq, 0.0).astype(np.float32)
    # lhsT for carry rank-1: a^(p+1)
    apow_rel = (A ** (p + 1.0))[None, :].astype(np.float32)
    # lhsT for x broadcast rank-1
    ones1 = np.ones((1, 128), dtype=np.float32)
    # per-partition output scale: apow_abs[p, kb] = a^(kb*128 + p + 1)
    kb = np.arange(NKB, dtype=np.float64)
    apow_abs = (A ** (kb[None, :] * 128.0 + p[:, None] + 1.0)).astype(np.float32)
    c = {
        "mtriT": mtriT,
        "apow_rel": apow_rel,
        "ones1": ones1,
        "apow_abs": apow_abs,
    }
    _cache["consts"] = c
    return c


def _build_nc():
    if "nc" in _cache:
        return _cache["nc"]
    nc = bacc.Bacc(
        "TRN2", target_bir_lowering=False, debug=False, num_devices=NCORES
    )
    x_p = nc.declare_dram_parameter("x", [BPC, LENGTH], F32, isOutput=False)
    z_p = nc.declare_dram_parameter("noise", [NK, LENGTH], F32, isOutput=False)
    mtriT_p = nc.declare_dram_parameter("mtriT", [128, 128], F32, isOutput=False)
    apr_p = nc.declare_dram_parameter("apow_rel", [1, 128], F32, isOutput=False)
    ones_p = nc.declare_dram_parameter("ones1", [1, 128], F32, isOutput=False)
    apa_p = nc.declare_dram_parameter("apow_abs", [128, NKB], F32, isOutput=False)
    out_p = nc.declare_dram_parameter("out", [BPC, STEPS, LENGTH], F32, isOutput=True)

    HALF = 512  # fp32 matmul moving-free-dim limit

    with tile.TileContext(nc) as tc:
        with (
            tc.tile_pool(name="consts", bufs=1) as consts,
            tc.tile_pool(name="pers", bufs=1) as pers,
            tc.tile_pool(name="zt", bufs=3) as ztp,
            tc.tile_pool(name="outp", bufs=8) as outp,
            tc.tile_pool(name="psc", bufs=2, space="PSUM") as pscp,
            tc.tile_pool(name="psx", bufs=1, space="PSUM") as psxp,
            tc.tile_pool(name="pscy", bufs=1, space="PSUM") as cyp,
        ):
            # ---- constants in ----
            mtriT = consts.tile([128, 128], F32, tag="mtriT")
            nc.sync.dma_start(out=mtriT[:], in_=mtriT_p[:])
            apr = consts.tile([1, 128], F32, tag="apr")
            nc.sync.dma_start(out=apr[:], in_=apr_p[:])
            ones1 = consts.tile([1, 128], F32, tag="ones1")
            nc.sync.dma_start(out=ones1[:], in_=ones_p[:])
            apa = consts.tile([128, NKB], F32, tag="apa")
            nc.sync.dma_start(out=apa[:], in_=apa_p[:])
            xt = consts.tile([BPC, LENGTH], F32, tag="xt")
            nc.sync.dma_start(out=xt[:], in_=x_p[:])

            # ---- k=0 plane: out[:, 0, :] = x ----
            nc.sync.dma_start(out=out_p[:, 0, :], in_=xt[:])

            # ---- persistent tiles ----
            # x rows as separate 1-partition tiles (matmul rhs needs base partition 0)
            xrow = [
                pers.tile([1, LENGTH], F32, tag=f"xr{b}", name=f"xr{b}")
                for b in range(BPC)
            ]
            for b in range(BPC):
                nc.sync.dma_start(out=xrow[b][:], in_=x_p[b : b + 1, :])

            xb = [
                pers.tile([128, LENGTH], F32, tag=f"xb{b}", name=f"xb{b}")
                for b in range(BPC)
            ]
            ct = [
                pers.tile([128, LENGTH], F32, tag=f"c{k}", name=f"c{k}")
                for k in range(NKB)
            ]
            carry = [
                pers.tile([1, LENGTH], F32, tag=f"cy{k}", name=f"cy{k}")
                for k in range(NKB - 1)
            ]

            def emit_xb(b):
                # broadcast x row b to 128 partitions: ones1.T @ x[b, :]
                ps = psxp.tile([128, LENGTH], F32, tag="psx")
                for h in range(LENGTH // HALF):
                    sl = slice(h * HALF, (h + 1) * HALF)
                    nc.tensor.matmul(
                        ps[:, sl], ones1[:, :], xrow[b][:, sl],
                        start=True, stop=True,
                    )
                nc.scalar.activation(
                    xb[b][:], ps[:], mybir.ActivationFunctionType.Copy
                )

            for b in range(BPC):
                emit_xb(b)

            for kb in range(NKB):
                rows = min(128, NK - kb * 128)  # 128, ..., 103 for last
                k0 = 1 + kb * 128
                zt = ztp.tile([128, LENGTH], F32, tag="zt")
                nc.sync.dma_start(
                    out=zt[:rows, :], in_=z_p[kb * 128 : kb * 128 + rows, :]
                )
                if kb < NKB - 1:
                    # carry chain: carry[kb] = sum_q b*a^(127-q)*z[q,:] + a^128*carry[kb-1]
                    # (mtriT[:,127] is exactly the b*a^(127-q) column; apr[0,127] = a^128)
                    cps = cyp.tile([1, LENGTH], F32, tag="cps", name="cps")
                    for h in range(LENGTH // HALF):
                        sl = slice(h * HALF, (h + 1) * HALF)
                        nc.tensor.matmul(
                            cps[:1, sl], mtriT[:128, 127:128], zt[:128, sl],
                            start=True, stop=(kb == 0),
                        )
                        if kb > 0:
                            nc.tensor.matmul(
                                cps[:1, sl], apr[0:1, 127:128],
                                carry[kb - 1][:1, sl],
                                start=False, stop=True,
                            )
                    nc.scalar.activation(
                        carry[kb][:], cps[:1, :],
                        mybir.ActivationFunctionType.Copy,
                    )
                ps = pscp.tile([128, LENGTH], F32, tag="psc")
                for h in range(LENGTH // HALF):
                    sl = slice(h * HALF, (h + 1) * HALF)
                    # in-block triangular accumulation
                    nc.tensor.matmul(
                        ps[:rows, sl], mtriT[:rows, :rows], zt[:rows, sl],
                        start=True, stop=(kb == 0),
                    )
                    if kb > 0:
                        # + a^(p+1) * c[end of previous block]
                        nc.tensor.matmul(
                            ps[:rows, sl], apr[:, :rows], carry[kb - 1][:, sl],
                            start=False, stop=True,
                        )
                nc.scalar.activation(
                    ct[kb][:rows, :], ps[:rows, :],
                    mybir.ActivationFunctionType.Copy,
                )
                for b in range(BPC):
                    ot = outp.tile([128, LENGTH], F32, tag="ot")
                    # out = (xb * a^k) + c   in one DVE op
                    nc.vector.scalar_tensor_tensor(
                        ot[:rows, :],
                        xb[b][:rows, :],
                        apa[:rows, kb : kb + 1],
                        ct[kb][:rows, :],
                        mybir.AluOpType.mult,
                        mybir.AluOpType.add,
                    )
                    nc.sync.dma_start(
                        out=out_p[b, k0 : k0 + rows, :], in_=ot[:rows, :]
                    )

    nc.compile()
    _cache["nc"] = nc
    return nc


def kernel(x: np.ndarray, noise: np.ndarray) -> np.ndarray:
    x = np.ascontiguousarray(np.asarray(x), dtype=np.float32)
    noise = np.ascontiguousarray(np.asarray(noise), dtype=np.float32)
    assert x.shape == (BATCH, LENGTH) and noise.shape == (NK, LENGTH)

    nc = _build_nc()
    consts = _consts()
    in_maps = []
    for c in range(NCORES):
        m = dict(consts)
        m["noise"] = noise
        m["x"] = x[c * BPC : (c + 1) * BPC]
        in_maps.append(m)

    res = run_bass_kernel_spmd(nc, in_maps, core_ids=list(range(NCORES)))
    _cache["last_result"] = res
    out = np.concatenate([res.results[i]["out"] for i in range(NCORES)], axis=0)
    return out


def last_exec_time_ns():
    r = _cache.get("last_result")
    return None if r is None else r.exec_time_ns


# revision 17
# speedup vs baseline: 2.2511x; 2.2511x over previous
"""ForwardDiffusion (Ornstein-Uhlenbeck Euler-Maruyama) Trainium2 kernel.

Math: x_k = a*x_{k-1} + b*z_k with a = 1-THETA*DT, b = SIGMA0*sqrt(DT).
Closed form: x_k = a^k * x0 + c_k where c_k = sum_{j<=k} a^{k-j} * b * z_j
depends only on the (batch-shared) noise. Each core handles 8 batch rows:
  - c tiles (k blocks of 128 on partitions) via triangular matmul on PE
    (bf16 in, f32 accumulate), with rank-1 carry propagation between blocks.
  - out tile (b, block pair) = (xb_bcast * a^k) + c  via fused DVE ops,
    streamed to DRAM as 1MB contiguous writes. Memory-bound: ~33MB/core.
Data parallel over batch: x sharded 8 ways, noise replicated, no collectives.
Noise is zero-padded to 1024 rows on host so every tile is a full 128
partitions; out is (8,1025,1024) per core, sliced to 1000 steps on host.
"""

import math
import os

import numpy as np
import ml_dtypes

import concourse.bass as bass
import concourse.bacc as bacc
import concourse.mybir as mybir
import concourse.tile as tile
from concourse.bass_utils import run_bass_kernel_spmd

# Problem config (hardcoded per harness contract)
THETA = 1.0
SIGMA0 = 0.5
DT = 0.001
BATCH = 64
LENGTH = 1024
STEPS = 1000           # real output rows per batch element (k = 0..999)
NK = STEPS - 1         # real noise rows (k = 1..999)
NCORES = 8
BPC = BATCH // NCORES  # batch rows per core = 8
NKB = 8                # k blocks of 128 (padded)
NKPAD = NKB * 128      # 1024 padded noise rows
KROWS = 1 + NKPAD      # 1025 output rows per batch element on device

A = 1.0 - THETA * DT           # 0.999
B = SIGMA0 * math.sqrt(DT)     # 0.0158113883...

F32 = mybir.dt.float32
BF16 = mybir.dt.bfloat16
NP_BF16 = ml_dtypes.bfloat16

_cache = {}


def _consts():
    """Host-precomputed constant tensors (exact in f64, then cast)."""
    if "consts" in _cache:
        return _cache["consts"]
    p = np.arange(128, dtype=np.float64)
    # lhsT for in-block triangular matmul: mtriT[q, p] = b * a^(p-q) for q <= p
    pq = p[None, :] - p[:, None]           # [q, p] -> p - q
    mtriT = np.where(pq >= 0, B * A ** pq, 0.0).astype(NP_BF16)
    # lhsT for carry rank-1: a^(p+1); [0,127] = a^128 reused as carry decay
    apow_rel = (A ** (p + 1.0))[None, :].astype(NP_BF16)
    # per-partition output scale: apow_abs[p, kb] = a^(kb*128 + p + 1)
    kb = np.arange(NKB, dtype=np.float64)
    apow_abs = (A ** (kb[None, :] * 128.0 + p[:, None] + 1.0)).astype(np.float32)
    c = {
        "mtriT": mtriT,
        "apow_rel": apow_rel,
        "apow_abs": apow_abs,
    }
    _cache["consts"] = c
    return c


def _build_nc():
    if "nc" in _cache:
        return _cache["nc"]
    nc = bacc.Bacc(
        "TRN2", target_bir_lowering=False, debug=False, num_devices=NCORES
    )
    x_p = nc.declare_dram_parameter("x", [BPC, LENGTH], F32, isOutput=False)
    z_p = nc.declare_dram_parameter("noise", [NKPAD, LENGTH], BF16, isOutput=False)
    mtriT_p = nc.declare_dram_parameter("mtriT", [128, 128], BF16, isOutput=False)
    apr_p = nc.declare_dram_parameter("apow_rel", [1, 128], BF16, isOutput=False)
    apa_p = nc.declare_dram_parameter("apow_abs", [128, NKB], F32, isOutput=False)
    out_p = nc.declare_dram_parameter("out", [BPC, KROWS, LENGTH], F32, isOutput=True)

    HALF = 512  # one PSUM bank of f32 per matmul

    with tile.TileContext(nc) as tc:
        with (
            tc.tile_pool(name="consts", bufs=1) as consts,
            tc.tile_pool(name="pers", bufs=1) as pers,
            tc.tile_pool(name="zt", bufs=3) as ztp,
            tc.tile_pool(name="outp", bufs=6) as outp,
            tc.tile_pool(name="psc", bufs=2, space="PSUM") as pscp,
            tc.tile_pool(name="pscy", bufs=2, space="PSUM") as cyp,
        ):
            # ---- constants in ----
            mtriT = consts.tile([128, 128], BF16, tag="mtriT")
            nc.sync.dma_start(out=mtriT[:], in_=mtriT_p[:])
            apr = consts.tile([1, 128], BF16, tag="apr")
            nc.sync.dma_start(out=apr[:], in_=apr_p[:])
            apa = consts.tile([128, NKB], F32, tag="apa")
            nc.sync.dma_start(out=apa[:], in_=apa_p[:])
            xt = consts.tile([BPC, LENGTH], F32, tag="xt")
            nc.sync.dma_start(out=xt[:], in_=x_p[:])

            # ---- k=0 plane: out[:, 0, :] = x ----
            nc.sync.dma_start(out=out_p[:, 0, :], in_=xt[:])

            # ---- x rows broadcast to 128 partitions via stride-0 DMA ----
            xb = [
                pers.tile([128, LENGTH], F32, tag=f"xb{b}", name=f"xb{b}")
                for b in range(BPC)
            ]
            for b in range(BPC):
                nc.sync.dma_start(
                    out=xb[b][:],
                    in_=x_p[b : b + 1, :].broadcast_to((128, LENGTH)),
                )

            ct = [
                pers.tile([128, LENGTH], F32, tag=f"c{k}", name=f"c{k}")
                for k in range(NKB)
            ]
            carry = [
                pers.tile([1, LENGTH], BF16, tag=f"cy{k}", name=f"cy{k}")
                for k in range(NKB - 1)
            ]

            for kb in range(NKB):
                zt = ztp.tile([128, LENGTH], BF16, tag="zt")
                nc.sync.dma_start(out=zt[:], in_=z_p[kb * 128 : (kb + 1) * 128, :])
                if kb < NKB - 1:
                    # carry chain: carry[kb] = sum_q b*a^(127-q)*z[q,:] + a^128*carry[kb-1]
                    # (mtriT[:,127] is exactly the b*a^(127-q) column; apr[0,127] = a^128)
                    cps = cyp.tile([1, LENGTH], F32, tag="cps", name="cps")
                    for h in range(LENGTH // HALF):
                        sl = slice(h * HALF, (h + 1) * HALF)
                        nc.tensor.matmul(
                            cps[:1, sl], mtriT[:128, 127:128], zt[:, sl],
                            start=True, stop=(kb == 0),
                        )
                        if kb > 0:
                            nc.tensor.matmul(
                                cps[:1, sl], apr[0:1, 127:128],
                                carry[kb - 1][:1, sl],
                                start=False, stop=True,
                            )
                    nc.scalar.activation(
                        carry[kb][:], cps[:1, :],
                        mybir.ActivationFunctionType.Copy,
                    )
                ps = pscp.tile([128, LENGTH], F32, tag="psc")
                for h in range(LENGTH // HALF):
                    sl = slice(h * HALF, (h + 1) * HALF)
                    # in-block triangular accumulation
                    nc.tensor.matmul(
                        ps[:, sl], mtriT[:, :], zt[:, sl],
                        start=True, stop=(kb == 0),
                    )
                    if kb > 0:
                        # + a^(p+1) * c[end of previous block]
                        nc.tensor.matmul(
                            ps[:, sl], apr[:, :], carry[kb - 1][:, sl],
                            start=False, stop=True,
                        )
                nc.scalar.activation(
                    ct[kb][:], ps[:], mybir.ActivationFunctionType.Copy
                )

                if kb % 2 == 1:
                    # pair sweep: blocks (kb-1, kb) -> one 1MB DMA per batch row
                    kb0 = kb - 1
                    for b in range(BPC):
                        ot = outp.tile([128, 2 * LENGTH], F32, tag="ot")
                        for j, kbx in enumerate((kb0, kb)):
                            nc.vector.scalar_tensor_tensor(
                                ot[:, j * LENGTH : (j + 1) * LENGTH],
                                xb[b][:, :],
                                apa[:, kbx : kbx + 1],
                                ct[kbx][:, :],
                                mybir.AluOpType.mult,
                                mybir.AluOpType.add,
                            )
                        dst = out_p[
                            b, 1 + kb0 * 128 : 1 + kb0 * 128 + 256, :
                        ].rearrange("(c p) l -> p c l", p=128)
                        src = ot[:, :].rearrange("p (c l) -> p c l", l=LENGTH)
                        nc.sync.dma_start(out=dst, in_=src)

    nc.compile()
    _cache["nc"] = nc
    return nc


def kernel(x: np.ndarray, noise: np.ndarray) -> np.ndarray:
    x = np.ascontiguousarray(np.asarray(x), dtype=np.float32)
    noise = np.asarray(noise)
    assert x.shape == (BATCH, LENGTH) and noise.shape == (NK, LENGTH)

    zpad = np.zeros((NKPAD, LENGTH), dtype=NP_BF16)
    zpad[:NK] = noise.astype(NP_BF16)

    nc = _build_nc()
    consts = _consts()
    in_maps = []
    for c in range(NCORES):
        m = dict(consts)
        m["noise"] = zpad
        m["x"] = x[c * BPC : (c + 1) * BPC]
        in_maps.append(m)

    res = run_bass_kernel_spmd(nc, in_maps, core_ids=list(range(NCORES)))
    _cache["last_result"] = res
    out = np.concatenate(
        [res.results[i]["out"][:, :STEPS, :] for i in range(NCORES)], axis=0
    )
    return np.ascontiguousarray(out)


def last_exec_time_ns():
    r = _cache.get("last_result")
    return None if r is None else r.exec_time_ns


# revision 18
# speedup vs baseline: 2.6391x; 1.1724x over previous
"""ForwardDiffusion (Ornstein-Uhlenbeck Euler-Maruyama) Trainium2 kernel.

Math: x_k = a*x_{k-1} + b*z_k with a = 1-THETA*DT, b = SIGMA0*sqrt(DT).
Closed form: x_k = a^k * x0 + c_k where c_k = sum_{j<=k} a^{k-j} * b * z_j
depends only on the (batch-shared) noise. Each core handles 8 batch rows:
  - c tiles (k blocks of 128 on partitions) via triangular matmul on PE
    (bf16 in, f32 accumulate), with rank-1 carry propagation between blocks.
  - out tile (batch pair, k block) = (x_bcast * a^k) + c in ONE fused DVE op
    (c broadcast along the free dim), streamed to DRAM as 1MB writes.
Inputs ride the Activation-engine HWDGE ring; outputs ride the SP ring, so
the noise loads never queue behind the 1MB output stream.
Data parallel over batch: x sharded 8 ways, noise replicated, no collectives.
Noise is zero-padded to 1024 rows on host so every tile is a full 128
partitions; out is (8,1025,1024) per core, sliced to 1000 steps on host.
"""

import math
import os

import numpy as np
import ml_dtypes

import concourse.bass as bass
import concourse.bacc as bacc
import concourse.mybir as mybir
import concourse.tile as tile
from concourse.bass_utils import run_bass_kernel_spmd

# Problem config (hardcoded per harness contract)
THETA = 1.0
SIGMA0 = 0.5
DT = 0.001
BATCH = 64
LENGTH = 1024
STEPS = 1000           # real output rows per batch element (k = 0..999)
NK = STEPS - 1         # real noise rows (k = 1..999)
NCORES = 8
BPC = BATCH // NCORES  # batch rows per core = 8
NPAIR = BPC // 2       # batch pairs per core = 4
NKB = 8                # k blocks of 128 (padded)
NKPAD = NKB * 128      # 1024 padded noise rows
KROWS = 1 + NKPAD      # 1025 output rows per batch element on device

A = 1.0 - THETA * DT           # 0.999
B = SIGMA0 * math.sqrt(DT)     # 0.0158113883...

F32 = mybir.dt.float32
BF16 = mybir.dt.bfloat16
NP_BF16 = ml_dtypes.bfloat16

_cache = {}


def _consts():
    """Host-precomputed constant tensors (exact in f64, then cast)."""
    if "consts" in _cache:
        return _cache["consts"]
    p = np.arange(128, dtype=np.float64)
    # lhsT for in-block triangular matmul: mtriT[q, p] = b * a^(p-q) for q <= p
    pq = p[None, :] - p[:, None]           # [q, p] -> p - q
    mtriT = np.where(pq >= 0, B * A ** pq, 0.0).astype(NP_BF16)
    # lhsT for carry rank-1: a^(p+1); [0,127] = a^128 reused as carry decay
    apow_rel = (A ** (p + 1.0))[None, :].astype(NP_BF16)
    # per-partition output scale: apow_abs[p, kb] = a^(kb*128 + p + 1)
    kb = np.arange(NKB, dtype=np.float64)
    apow_abs = (A ** (kb[None, :] * 128.0 + p[:, None] + 1.0)).astype(np.float32)
    c = {
        "mtriT": mtriT,
        "apow_rel": apow_rel,
        "apow_abs": apow_abs,
    }
    _cache["consts"] = c
    return c


def _build_nc():
    if "nc" in _cache:
        return _cache["nc"]
    nc = bacc.Bacc(
        "TRN2", target_bir_lowering=False, debug=False, num_devices=NCORES
    )
    x_p = nc.declare_dram_parameter("x", [BPC, LENGTH], F32, isOutput=False)
    z_p = nc.declare_dram_parameter("noise", [NKPAD, LENGTH], BF16, isOutput=False)
    mtriT_p = nc.declare_dram_parameter("mtriT", [128, 128], BF16, isOutput=False)
    apr_p = nc.declare_dram_parameter("apow_rel", [1, 128], BF16, isOutput=False)
    apa_p = nc.declare_dram_parameter("apow_abs", [128, NKB], F32, isOutput=False)
    out_p = nc.declare_dram_parameter("out", [BPC, KROWS, LENGTH], F32, isOutput=True)

    HALF = 512  # one PSUM bank of f32 per matmul
    L2 = 2 * LENGTH

    with tile.TileContext(nc) as tc:
        with (
            tc.tile_pool(name="consts", bufs=1) as consts,
            tc.tile_pool(name="pers", bufs=1) as pers,
            tc.tile_pool(name="zt", bufs=4) as ztp,
            tc.tile_pool(name="outp", bufs=6) as outp,
            tc.tile_pool(name="psc", bufs=2, space="PSUM") as pscp,
            tc.tile_pool(name="pscy", bufs=2, space="PSUM") as cyp,
        ):
            # ---- constants in (Activation HWDGE ring for all inputs) ----
            mtriT = consts.tile([128, 128], BF16, tag="mtriT")
            nc.scalar.dma_start(out=mtriT[:], in_=mtriT_p[:])
            apr = consts.tile([1, 128], BF16, tag="apr")
            nc.scalar.dma_start(out=apr[:], in_=apr_p[:])
            apa = consts.tile([128, NKB], F32, tag="apa")
            nc.scalar.dma_start(out=apa[:], in_=apa_p[:])
            xt = consts.tile([BPC, LENGTH], F32, tag="xt")
            nc.scalar.dma_start(out=xt[:], in_=x_p[:])

            # ---- k=0 plane: out[:, 0, :] = x (output -> SP ring) ----
            nc.sync.dma_start(out=out_p[:, 0, :], in_=xt[:])

            # persistent tiles
            xb2 = [
                pers.tile([128, L2], F32, tag=f"xb{i}", name=f"xb{i}")
                for i in range(NPAIR)
            ]
            ct = [
                pers.tile([128, LENGTH], F32, tag=f"c{k}", name=f"c{k}")
                for k in range(NKB)
            ]
            carry = [
                pers.tile([1, LENGTH], BF16, tag=f"cy{k}", name=f"cy{k}")
                for k in range(NKB - 1)
            ]

            zt = [None] * NKB

            def emit_zt(kb):
                t = ztp.tile([128, LENGTH], BF16, tag="zt")
                nc.scalar.dma_start(out=t[:], in_=z_p[kb * 128 : (kb + 1) * 128, :])
                zt[kb] = t

            def emit_xb(i):
                # both x rows of the pair broadcast to 128 partitions, one DMA
                src = (
                    x_p[2 * i : 2 * i + 2, :]
                    .rearrange("(u b) l -> u b l", u=1)
                    .broadcast_to((128, 2, LENGTH))
                )
                dst = xb2[i][:, :].rearrange("p (b l) -> p b l", l=LENGTH)
                nc.scalar.dma_start(out=dst, in_=src)

            # interleave noise-block loads with x broadcasts on the input ring
            emit_zt(0)
            emit_zt(1)
            for i in range(NPAIR):
                emit_xb(i)
                if i + 2 < NKB:
                    emit_zt(i + 2)
            for kb in range(NPAIR + 2, NKB):
                emit_zt(kb)

            for kb in range(NKB):
                if kb < NKB - 1:
                    # carry chain: carry[kb] = sum_q b*a^(127-q)*z[q,:] + a^128*carry[kb-1]
                    # (mtriT[:,127] is exactly the b*a^(127-q) column; apr[0,127] = a^128)
                    cps = cyp.tile([1, LENGTH], F32, tag="cps", name="cps")
                    for h in range(LENGTH // HALF):
                        sl = slice(h * HALF, (h + 1) * HALF)
                        nc.tensor.matmul(
                            cps[:1, sl], mtriT[:128, 127:128], zt[kb][:, sl],
                            start=True, stop=(kb == 0),
                        )
                        if kb > 0:
                            nc.tensor.matmul(
                                cps[:1, sl], apr[0:1, 127:128],
                                carry[kb - 1][:1, sl],
                                start=False, stop=True,
                            )
                    nc.scalar.activation(
                        carry[kb][:], cps[:1, :],
                        mybir.ActivationFunctionType.Copy,
                    )
                ps = pscp.tile([128, LENGTH], F32, tag="psc")
                for h in range(LENGTH // HALF):
                    sl = slice(h * HALF, (h + 1) * HALF)
                    # in-block triangular accumulation
                    nc.tensor.matmul(
                        ps[:, sl], mtriT[:, :], zt[kb][:, sl],
                        start=True, stop=(kb == 0),
                    )
                    if kb > 0:
                        # + a^(p+1) * c[end of previous block]
                        nc.tensor.matmul(
                            ps[:, sl], apr[:, :], carry[kb - 1][:, sl],
                            start=False, stop=True,
                        )
                nc.scalar.activation(
                    ct[kb][:], ps[:], mybir.ActivationFunctionType.Copy
                )

                # sweep: one (128, 2048) tile per batch pair = 1MB DMA
                k0 = 1 + kb * 128
                cbc = (
                    ct[kb][:, :]
                    .rearrange("p (u l) -> p u l", u=1)
                    .broadcast_to((128, 2, LENGTH))
                )
                for i in range(NPAIR):
                    ot = outp.tile([128, L2], F32, tag="ot")
                    o3 = ot[:, :].rearrange("p (b l) -> p b l", l=LENGTH)
                    i3 = xb2[i][:, :].rearrange("p (b l) -> p b l", l=LENGTH)
                    nc.vector.scalar_tensor_tensor(
                        o3,
                        i3,
                        apa[:, kb : kb + 1],
                        cbc,
                        mybir.AluOpType.mult,
                        mybir.AluOpType.add,
                    )
                    dst = out_p[
                        2 * i : 2 * i + 2, k0 : k0 + 128, :
                    ].rearrange("b k l -> k b l")
                    nc.sync.dma_start(out=dst, in_=o3)

    nc.compile()
    _cache["nc"] = nc
    return nc


def kernel(x: np.ndarray, noise: np.ndarray) -> np.ndarray:
    x = np.ascontiguousarray(np.asarray(x), dtype=np.float32)
    noise = np.asarray(noise)
    assert x.shape == (BATCH, LENGTH) and noise.shape == (NK, LENGTH)

    zpad = np.zeros((NKPAD, LENGTH), dtype=NP_BF16)
    zpad[:NK] = noise.astype(NP_BF16)

    nc = _build_nc()
    consts = _consts()
    in_maps = []
    for c in range(NCORES):
        m = dict(consts)
        m["noise"] = zpad
        m["x"] = x[c * BPC : (c + 1) * BPC]
        in_maps.append(m)

    res = run_bass_kernel_spmd(nc, in_maps, core_ids=list(range(NCORES)))
    _cache["last_result"] = res
    out = np.concatenate(
        [res.results[i]["out"][:, :STEPS, :] for i in range(NCORES)], axis=0
    )
    return np.ascontiguousarray(out)


def last_exec_time_ns():
    r = _cache.get("last_result")
    return None if r is None else r.exec_time_ns


# revision 19
# speedup vs baseline: 2.7113x; 1.0274x over previous
"""ForwardDiffusion (Ornstein-Uhlenbeck Euler-Maruyama) Trainium2 kernel.

Math: x_k = a*x_{k-1} + b*z_k with a = 1-THETA*DT, b = SIGMA0*sqrt(DT).
Closed form: x_k = a^k * x0 + c_k where c_k = sum_{j<=k} a^{k-j} * b * z_j
depends only on the (batch-shared) noise. Each core handles 8 batch rows:
  - c tiles (k blocks of 128 on partitions) via triangular matmul on PE
    (bf16 in, f32 accumulate), with rank-1 carry propagation between blocks.
  - out tile (batch pair, k block) = (x_bcast * a^k) + c in ONE fused DVE op
    (c broadcast along the free dim), streamed to DRAM as 1MB writes.
Inputs ride the Activation-engine HWDGE ring; outputs ride the SP ring, so
the noise loads never queue behind the 1MB output stream.
Data parallel over batch: x sharded 8 ways, noise replicated, no collectives.
Noise is zero-padded to 1024 rows on host so every tile is a full 128
partitions; out is (8,1025,1024) per core, sliced to 1000 steps on host.
"""

import math
import os

import numpy as np
import ml_dtypes

import concourse.bass as bass
import concourse.bacc as bacc
import concourse.mybir as mybir
import concourse.tile as tile
from concourse.bass_utils import run_bass_kernel_spmd

# Problem config (hardcoded per harness contract)
THETA = 1.0
SIGMA0 = 0.5
DT = 0.001
BATCH = 64
LENGTH = 1024
STEPS = 1000           # real output rows per batch element (k = 0..999)
NK = STEPS - 1         # real noise rows (k = 1..999)
NCORES = 8
BPC = BATCH // NCORES  # batch rows per core = 8
NPAIR = BPC // 2       # batch pairs per core = 4
NKB = 8                # k blocks of 128 (padded)
NKPAD = NKB * 128      # 1024 padded noise rows
KROWS = 1 + NKPAD      # 1025 output rows per batch element on device

A = 1.0 - THETA * DT           # 0.999
B = SIGMA0 * math.sqrt(DT)     # 0.0158113883...

F32 = mybir.dt.float32
BF16 = mybir.dt.bfloat16
NP_BF16 = ml_dtypes.bfloat16

_cache = {}


def _consts():
    """Host-precomputed constant tensors (exact in f64, then cast)."""
    if "consts" in _cache:
        return _cache["consts"]
    p = np.arange(128, dtype=np.float64)
    # lhsT for in-block triangular matmul: mtriT[q, p] = b * a^(p-q) for q <= p
    pq = p[None, :] - p[:, None]           # [q, p] -> p - q
    mtriT = np.where(pq >= 0, B * A ** pq, 0.0).astype(NP_BF16)
    # lhsT for carry rank-1: a^(p+1); [0,127] = a^128 reused as carry decay
    apow_rel = (A ** (p + 1.0))[None, :].astype(NP_BF16)
    # per-partition output scale: apow_abs[p, kb] = a^(kb*128 + p + 1)
    kb = np.arange(NKB, dtype=np.float64)
    apow_abs = (A ** (kb[None, :] * 128.0 + p[:, None] + 1.0)).astype(np.float32)
    c = {
        "mtriT": mtriT,
        "apow_rel": apow_rel,
        "apow_abs": apow_abs,
    }
    _cache["consts"] = c
    return c


def _build_nc():
    if "nc" in _cache:
        return _cache["nc"]
    nc = bacc.Bacc(
        "TRN2", target_bir_lowering=False, debug=False, num_devices=NCORES
    )
    x_p = nc.declare_dram_parameter("x", [BPC, LENGTH], F32, isOutput=False)
    z_p = nc.declare_dram_parameter("noise", [NKPAD, LENGTH], BF16, isOutput=False)
    mtriT_p = nc.declare_dram_parameter("mtriT", [128, 128], BF16, isOutput=False)
    apr_p = nc.declare_dram_parameter("apow_rel", [1, 128], BF16, isOutput=False)
    apa_p = nc.declare_dram_parameter("apow_abs", [128, NKB], F32, isOutput=False)
    out_p = nc.declare_dram_parameter("out", [BPC, KROWS, LENGTH], F32, isOutput=True)

    HALF = 512  # one PSUM bank of f32 per matmul
    L2 = 2 * LENGTH

    with tile.TileContext(nc) as tc:
        with (
            tc.tile_pool(name="consts", bufs=1) as consts,
            tc.tile_pool(name="pers", bufs=1) as pers,
            tc.tile_pool(name="zt", bufs=4) as ztp,
            tc.tile_pool(name="outp", bufs=6) as outp,
            tc.tile_pool(name="psc", bufs=2, space="PSUM") as pscp,
            tc.tile_pool(name="pscy", bufs=2, space="PSUM") as cyp,
        ):
            # ---- constants in (Activation HWDGE ring for all inputs) ----
            mtriT = consts.tile([128, 128], BF16, tag="mtriT")
            nc.scalar.dma_start(out=mtriT[:], in_=mtriT_p[:])

            zt = [None] * NKB

            def emit_zt(kb):
                t = ztp.tile([128, LENGTH], BF16, tag="zt")
                nc.scalar.dma_start(out=t[:], in_=z_p[kb * 128 : (kb + 1) * 128, :])
                zt[kb] = t

            emit_zt(0)  # right behind mtriT so the chain starts ASAP

            apr = consts.tile([1, 128], BF16, tag="apr")
            nc.scalar.dma_start(out=apr[:], in_=apr_p[:])
            apa = consts.tile([128, NKB], F32, tag="apa")
            nc.scalar.dma_start(out=apa[:], in_=apa_p[:])
            xt = consts.tile([BPC, LENGTH], F32, tag="xt")
            nc.scalar.dma_start(out=xt[:], in_=x_p[:])
            emit_zt(1)

            # ---- k=0 plane: out[:, 0, :] = x (output -> SP ring) ----
            nc.sync.dma_start(out=out_p[:, 0, :], in_=xt[:])

            # persistent tiles
            xb2 = [
                pers.tile([128, L2], F32, tag=f"xb{i}", name=f"xb{i}")
                for i in range(NPAIR)
            ]
            ct = [
                pers.tile([128, LENGTH], F32, tag=f"c{k}", name=f"c{k}")
                for k in range(NKB)
            ]
            carry = [
                pers.tile([1, LENGTH], BF16, tag=f"cy{k}", name=f"cy{k}")
                for k in range(NKB - 1)
            ]

            def emit_xb(i):
                # both x rows of the pair broadcast to 128 partitions, one
                # DMA on the GpSimd SWDGE ring (keeps the ACT ring short)
                src = (
                    x_p[2 * i : 2 * i + 2, :]
                    .rearrange("(u b) l -> u b l", u=1)
                    .broadcast_to((128, 2, LENGTH))
                )
                dst = xb2[i][:, :].rearrange("p (b l) -> p b l", l=LENGTH)
                nc.gpsimd.dma_start(out=dst, in_=src)

            for i in range(NPAIR):
                emit_xb(i)

            for kb in range(NKB):
                if kb + 2 < NKB:
                    emit_zt(kb + 2)
                if kb < NKB - 1:
                    # carry chain: carry[kb] = sum_q b*a^(127-q)*z[q,:] + a^128*carry[kb-1]
                    # (mtriT[:,127] is exactly the b*a^(127-q) column; apr[0,127] = a^128)
                    cps = cyp.tile([1, LENGTH], F32, tag="cps", name="cps")
                    for h in range(LENGTH // HALF):
                        sl = slice(h * HALF, (h + 1) * HALF)
                        nc.tensor.matmul(
                            cps[:1, sl], mtriT[:128, 127:128], zt[kb][:, sl],
                            start=True, stop=(kb == 0),
                        )
                        if kb > 0:
                            nc.tensor.matmul(
                                cps[:1, sl], apr[0:1, 127:128],
                                carry[kb - 1][:1, sl],
                                start=False, stop=True,
                            )
                    nc.scalar.activation(
                        carry[kb][:], cps[:1, :],
                        mybir.ActivationFunctionType.Copy,
                    )
                ps = pscp.tile([128, LENGTH], F32, tag="psc")
                for h in range(LENGTH // HALF):
                    sl = slice(h * HALF, (h + 1) * HALF)
                    # in-block triangular accumulation
                    nc.tensor.matmul(
                        ps[:, sl], mtriT[:, :], zt[kb][:, sl],
                        start=True, stop=(kb == 0),
                    )
                    if kb > 0:
                        # + a^(p+1) * c[end of previous block]
                        nc.tensor.matmul(
                            ps[:, sl], apr[:, :], carry[kb - 1][:, sl],
                            start=False, stop=True,
                        )
                nc.scalar.activation(
                    ct[kb][:], ps[:], mybir.ActivationFunctionType.Copy
                )

                # sweep: one (128, 2048) tile per batch pair = 1MB DMA
                k0 = 1 + kb * 128
                cbc = (
                    ct[kb][:, :]
                    .rearrange("p (u l) -> p u l", u=1)
                    .broadcast_to((128, 2, LENGTH))
                )
                for i in range(NPAIR):
                    ot = outp.tile([128, L2], F32, tag="ot")
                    o3 = ot[:, :].rearrange("p (b l) -> p b l", l=LENGTH)
                    i3 = xb2[i][:, :].rearrange("p (b l) -> p b l", l=LENGTH)
                    nc.vector.scalar_tensor_tensor(
                        o3,
                        i3,
                        apa[:, kb : kb + 1],
                        cbc,
                        mybir.AluOpType.mult,
                        mybir.AluOpType.add,
                    )
                    dst = out_p[
                        2 * i : 2 * i + 2, k0 : k0 + 128, :
                    ].rearrange("b k l -> k b l")
                    nc.sync.dma_start(out=dst, in_=o3)

    nc.compile()
    _cache["nc"] = nc
    return nc


def kernel(x: np.ndarray, noise: np.ndarray) -> np.ndarray:
    x = np.ascontiguousarray(np.asarray(x), dtype=np.float32)
    noise = np.asarray(noise)
    assert x.shape == (BATCH, LENGTH) and noise.shape == (NK, LENGTH)

    zpad = np.zeros((NKPAD, LENGTH), dtype=NP_BF16)
    zpad[:NK] = noise.astype(NP_BF16)

    nc = _build_nc()
    consts = _consts()
    in_maps = []
    for c in range(NCORES):
        m = dict(consts)
        m["noise"] = zpad
        m["x"] = x[c * BPC : (c + 1) * BPC]
        in_maps.append(m)

    res = run_bass_kernel_spmd(nc, in_maps, core_ids=list(range(NCORES)))
    _cache["last_result"] = res
    out = np.concatenate(
        [res.results[i]["out"][:, :STEPS, :] for i in range(NCORES)], axis=0
    )
    return np.ascontiguousarray(out)


def last_exec_time_ns():
    r = _cache.get("last_result")
    return None if r is None else r.exec_time_ns
